# revision 14
# baseline (speedup 1.0000x reference)
"""Distributed TRN2 Bass kernel for OpenFold-style gated attention with pair bias.

Problem: B=4, Q=K=1024, H=8 heads, D=32, C=256 (all fp32):
    q = (q_x @ wq.T)/sqrt(D);  k = kv_x @ wk.T;  v = kv_x @ wv.T
    a = softmax(q k^T + mask_bias + pair_bias)   (softmax over K)
    o = (a v) * sigmoid(q_x @ wg.T + bg)
    out = o @ wo.T + bo

Sharding: 8 cores = (batch b, query-half qh); no collectives; host
concatenates per-core outputs.

v3 dataflow (per core; all feature-on-partitions, no on-device transposes):
  - the PE computes ONLY q.k^T scores (transposed, [k-part, q-free]);
    pair_bias never touches the tensor engine;
  - exp(qk + pb) is factorized exp(qk)*exp(pb) and split per
    (t4, j, pair) unit between two paths:
      * sch units (DVE): pair_bias ships as A*pb bf16 and one fused
        affine_then_add computes int16(round(A*qk + B + A*pb)) whose bits
        ARE the bf16 Schraudolph approximation of exp(qk+pb) -- one DVE
        op per unit, pb-add included;
      * ACT units: pair_bias ships as exp(pb) bf16; ACT's hardware Exp
        LUT computes exp(qk) (|qk| <~ 0.6) and an all-bf16 multiply
        (GpSimd or DVE at 2x rate) forms exp(qk)*exp(pb);
    the per-unit choice is a compile-time schedule and the host packs the
    matching pb encoding into one [t4, j, p, h4, q] bf16 tensor, streamed
    on both hardware DGE queues (sync + scalar);
  - AV + denominator matmuls column-packed (denominator via
    32x-replicated u = exp(mask_bias) lhsT, so normalization is pure
    elementwise; den consumes the same p tiles, canceling the sch
    approximation's scale error); gating multiplies run on DVE/GpSimd
    straight out of PSUM; output projection at the tail (score PSUM pool
    triple-buffered during the loop).
"""

import numpy as np

H, D, C = 8, 32, 256
B, Q, K = 4, 1024, 1024
QL = 512  # queries per core
NCORES = 8
P = 128
NKC = K // P  # 8 k-chunks of 128

SCH_A = 184.6649652337873  # 128 * log2(e)
SCH_B = 16248.8


def _sch_unit(t4, j, pair):
    """True -> DVE schraudolph unit (15 of 32), else ACT exp * mul unit."""
    if (j + pair) % 2 == 0:
        return False
    return not (t4 == 1 and j == 5)


def _mul_on_dve(n):
    """For ACT units: 5 of 17 multiplies on DVE (2x bf16), rest on GpSimd."""
    return n % 17 in (0, 4, 7, 11, 14)


_CACHE = {}
LAST_RESULTS = None


def _build_nc():
    from contextlib import ExitStack

    from concourse import bacc, mybir, tile

    f32 = mybir.dt.float32
    bf16 = mybir.dt.bfloat16
    i16 = mybir.dt.int16
    EXP = mybir.ActivationFunctionType.Exp
    SIG = mybir.ActivationFunctionType.Sigmoid
    IDENT = mybir.ActivationFunctionType.Identity

    nc = bacc.Bacc("TRN2", target_bir_lowering=False, debug=False, num_devices=NCORES)

    CB = 5888  # bf16 constant-blob columns
    pbm_d = nc.dram_tensor("pbm", [2, NKC, P, 4, QL], bf16, kind="ExternalInput").ap()
    cb_d = nc.dram_tensor("cb", [P, CB], bf16, kind="ExternalInput").ap()
    cf_d = nc.dram_tensor("cf", [P, 12], f32, kind="ExternalInput").ap()
    out_d = nc.dram_tensor("out", [C, QL], f32, kind="ExternalOutput").ap()

    with tile.TileContext(nc) as tc, ExitStack() as ctx:
        # ---- persistent tiles -------------------------------------------
        cp = ctx.enter_context(tc.tile_pool(name="const", bufs=1))

        def ptile(shape, dtype, name):
            return cp.tile(shape, dtype, name=name, tag=name)

        cb_sb = ptile([P, CB], bf16, "cb_sb")
        cf_sb = ptile([P, 12], f32, "cf_sb")

        def cbv(lo, hi, a=None):
            v = cb_sb[:, lo:hi]
            return v.rearrange("p (a b) -> p a b", a=a) if a else v

        wk_bf = cbv(0, 512, 2)        # [128, 2, 256]
        kv_bf = cbv(512, 2560, 2)     # [128, 2, 1024]
        wq_bf = cbv(2560, 3072, 2)
        qx_bf = cbv(3072, 4096, 2)    # [128, 2, 512]
        wv_bf = cbv(4096, 4608, 2)
        wg_bf = cbv(4608, 5120, 2)
        woB_bf = cbv(5120, 5632, 2)   # [hd-in-half, half t4, c]
        u32_bf = cbv(5632, 5888, NKC)  # [128, 8, 32]: u replicated 32x
        bgT_sb = cf_sb[:, 0:2]
        u_sb = cf_sb[:, 2:2 + NKC]  # f32 u = exp(mask_bias), host-computed
        boT_sb = cf_sb[:, 10:12]

        qT_bf = ptile([P, 2, QL], bf16, "qT_bf")  # [hd-part, t, q]
        kT_bf = ptile([P, 2, K], bf16, "kT_bf")  # [hd-part, t, k]
        v1_bf = ptile([P, NKC, C], bf16, "v1_bf")  # v * u, [k-part, chunk, hd]
        g_bf = ptile([P, 2, QL], bf16, "g_bf")  # sigmoid gate, stacked halves
        og_bf = ptile([P, 2, QL], bf16, "og_bf")  # gated+normalized o^T

        # constants split across all three DMA queues by priority; pair-bias
        # tiles round-robin the same three queues (aggregate BW ~2.5x single
        # queue). Rolling prefetch 3 deep (pool bufs=4).
        nc.sync.dma_start(out=cb_sb[:, 0:2560], in_=cb_d[:, 0:2560])
        nc.scalar.dma_start(out=cb_sb[:, 2560:4096], in_=cb_d[:, 2560:4096])
        nc.gpsimd.dma_start(out=cb_sb[:, 4096:CB], in_=cb_d[:, 4096:CB])
        nc.gpsimd.dma_start(out=cf_sb[:], in_=cf_d[:])

        pb_pool = ctx.enter_context(tc.tile_pool(name="pb", bufs=4))
        pb_order = [(t4, j) for t4 in range(2) for j in range(NKC)]
        pbt = {}
        pb_engs = [nc.sync, nc.scalar, nc.gpsimd]

        def issue_pb(idx):
            if idx < len(pb_order):
                t4, j = pb_order[idx]
                t = pb_pool.tile([P, 4, QL], bf16, tag="pb", name=f"pb{t4}_{j}")
                pb_engs[idx % 3].dma_start(out=t[:], in_=pbm_d[t4, j])
                pbt[(t4, j)] = t

        for i in range(3):
            issue_pb(i)

        # ---- stage 1: projections ---------------------------------------
        with tc.tile_pool(name="ps1", bufs=3, space="PSUM") as ps1:
            # kT[f, k] first (the attention critical path), then qT[f, q]
            for t in range(2):
                for fc in range(2):
                    ps = ps1.tile([P, QL], f32, tag="ps1")
                    for ci in range(2):
                        nc.tensor.matmul(
                            ps[:],
                            lhsT=wk_bf[:, ci, t * P:(t + 1) * P],
                            rhs=kv_bf[:, ci, fc * QL:(fc + 1) * QL],
                            start=(ci == 0),
                            stop=(ci == 1),
                        )
                    nc.scalar.activation(
                        kT_bf[:, t, fc * QL:(fc + 1) * QL], ps[:],
                        mybir.ActivationFunctionType.Copy,
                    )

            for t in range(2):
                ps = ps1.tile([P, QL], f32, tag="ps1")
                for ci in range(2):
                    nc.tensor.matmul(
                        ps[:],
                        lhsT=wq_bf[:, ci, t * P:(t + 1) * P],
                        rhs=qx_bf[:, ci, :],
                        start=(ci == 0),
                        stop=(ci == 1),
                    )
                nc.vector.tensor_copy(qT_bf[:, t, :], ps[:])

            # v per k-chunk, scaled per-partition by u = exp(mask_bias)
            for j in range(NKC):
                ps = ps1.tile([P, C], f32, tag="ps1")
                for ci in range(2):
                    nc.tensor.matmul(
                        ps[:],
                        lhsT=kv_bf[:, ci, j * P:(j + 1) * P],
                        rhs=wv_bf[:, ci, :],
                        start=(ci == 0),
                        stop=(ci == 1),
                    )
                nc.scalar.activation(
                    v1_bf[:, j, :], ps[:], mybir.ActivationFunctionType.Copy,
                    bias=0.0, scale=u_sb[:, j:j + 1],
                )

            # gate halves: g = sigmoid(wg x + bg), stacked [128=4 heads x 32d]
            for t in range(2):
                ps = ps1.tile([P, QL], f32, tag="ps1")
                for ci in range(2):
                    nc.tensor.matmul(
                        ps[:],
                        lhsT=wg_bf[:, ci, t * P:(t + 1) * P],
                        rhs=qx_bf[:, ci, :],
                        start=(ci == 0),
                        stop=(ci == 1),
                    )
                nc.scalar.activation(
                    g_bf[:, t, :], ps[:], SIG, bias=bgT_sb[:, t:t + 1]
                )

        # ---- stage 2: attention, 2 groups of 4 column-packed heads ------
        mul_n = 0
        with tc.tile_pool(name="pp", bufs=4) as p_pool, tc.tile_pool(
            name="eq", bufs=3
        ) as eq_pool, tc.tile_pool(name="nrm", bufs=2) as nrm, tc.tile_pool(
            name="ps_s", bufs=3, space="PSUM"
        ) as ps_s, tc.tile_pool(name="ps_o", bufs=1, space="PSUM") as ps_o, tc.tile_pool(
            name="ps_d", bufs=1, space="PSUM"
        ) as ps_d:
            for t4 in range(2):
                o_ps = ps_o.tile([P, QL], f32, tag="ps_o")
                d_ps = ps_d.tile([P, QL], f32, tag="ps_d")
                p2s = {}

                def emit_av(pj):
                    pp2 = p2s[pj]
                    for h4 in range(4):
                        co = h4 * D
                        nc.tensor.matmul(
                            o_ps[co:co + D, :],
                            lhsT=v1_bf[
                                :, pj, (t4 * 4 + h4) * D:(t4 * 4 + h4 + 1) * D
                            ],
                            rhs=pp2[:, h4, :],
                            start=(pj == 0),
                            stop=(pj == NKC - 1),
                            tile_position=(0, co),
                            skip_group_check=True,
                        )
                    for h4 in range(4):
                        co = h4 * D
                        nc.tensor.matmul(
                            d_ps[co:co + D, :],
                            lhsT=u32_bf[:, pj, :],
                            rhs=pp2[:, h4, :],
                            start=(pj == 0),
                            stop=(pj == NKC - 1),
                            tile_position=(0, co),
                            skip_group_check=True,
                        )

                for j in range(NKC):
                    issue_pb(pb_order.index((t4, j)) + 3)
                    pt = pbt[(t4, j)]
                    p2 = p_pool.tile([P, 4, QL], bf16, tag="p2")
                    p2s[j] = p2
                    for pair in range(2):
                        s2 = ps_s.tile([P, 2, QL], f32, tag="ps_s")
                        for hh in range(2):
                            h = 2 * pair + hh
                            pr = h * D
                            nc.tensor.matmul(
                                s2[:, hh, :],
                                lhsT=kT_bf[pr:pr + D, t4, j * P:(j + 1) * P],
                                rhs=qT_bf[pr:pr + D, t4, :],
                                start=True,
                                stop=True,
                                tile_position=(pr, 0),
                                skip_group_check=True,
                            )
                        pslice = p2[:, 2 * pair:2 * pair + 2, :]
                        pbslice = pt[:, 2 * pair:2 * pair + 2, :]
                        if _sch_unit(t4, j, pair):
                            nc.vector.affine_then_add(
                                pslice.bitcast(i16), s2[:], pbslice,
                                SCH_A, SCH_B,
                            )
                        else:
                            eqk = eq_pool.tile([P, 2, QL], bf16, tag="eqk")
                            nc.scalar.activation(eqk[:], s2[:], EXP)
                            eng = nc.vector if _mul_on_dve(mul_n) else nc.gpsimd
                            mul_n += 1
                            eng.tensor_mul(pslice, eqk[:], pbslice)
                    if j >= 2:
                        emit_av(j - 2)  # 2-deep: absorbs the exp+mul latency
                emit_av(NKC - 2)
                emit_av(NKC - 1)
                # normalize + gate: rb = 1/den (DVE, from PSUM); ge = g*rb
                # (gpsimd, SBUF-only); og = o_ps * ge (DVE, from PSUM)
                rbt = nrm.tile([P, QL], f32, tag="rbt")
                nc.vector.reciprocal_approx_fast(rbt[:], d_ps[:])
                ge = nrm.tile([P, QL], bf16, tag="ge")
                nc.gpsimd.tensor_mul(ge[:], g_bf[:, t4, :], rbt[:])
                nc.vector.tensor_mul(og_bf[:, t4, :], o_ps[:], ge[:])

        # ---- tail: output projection ------------------------------------
        with tc.tile_pool(name="ps_out", bufs=2, space="PSUM") as ps_out_pool, \
                tc.tile_pool(name="otail", bufs=2) as otail:
            for t in range(2):
                ps = ps_out_pool.tile([P, QL], f32, tag="ps_out", name=f"po{t}")
                for t4 in range(2):
                    nc.tensor.matmul(
                        ps[:],
                        lhsT=woB_bf[:, t4, t * P:(t + 1) * P],
                        rhs=og_bf[:, t4, :],
                        start=(t4 == 0),
                        stop=(t4 == 1),
                    )
                o_out = otail.tile([P, QL], f32, tag="o_out", name=f"oo{t}")
                nc.scalar.activation(
                    o_out[:], ps[:], IDENT, bias=boT_sb[:, t:t + 1]
                )
                nc.sync.dma_start(out=out_d[t * P:(t + 1) * P, :], in_=o_out[:])

    nc.compile()
    return nc


def _get_nc():
    if "nc" not in _CACHE:
        _CACHE["nc"] = _build_nc()
    return _CACHE["nc"]


def _make_in_maps(q_x, kv_x, mask_bias, pair_bias, wq, wk, wv, wg, bg, wo, bo):
    f = np.float32
    q_x = np.asarray(q_x, f)
    kv_x = np.asarray(kv_x, f)
    mask_bias = np.asarray(mask_bias, f)
    pair_bias = np.asarray(pair_bias, f)
    wq = np.asarray(wq, f)
    wk = np.asarray(wk, f)
    wv = np.asarray(wv, f)
    wg = np.asarray(wg, f)
    bg = np.asarray(bg, f)
    wo = np.asarray(wo, f)
    bo = np.asarray(bo, f)

    import ml_dtypes
    bf = ml_dtypes.bfloat16

    def part_major(x, cols):  # [256, cols] -> [128, 2, cols] partition-major
        return x.reshape(2, P, cols).transpose(1, 0, 2)

    CB = 5888
    cb = np.zeros((P, CB), bf)
    cb[:, 0:512] = part_major(wk.T.astype(bf), C).reshape(P, 512)
    cb[:, 2560:3072] = part_major((wq / np.sqrt(D)).T.astype(bf), C).reshape(P, 512)
    cb[:, 4096:4608] = part_major(wv.T.astype(bf), C).reshape(P, 512)
    cb[:, 4608:5120] = part_major(wg.T.astype(bf), C).reshape(P, 512)
    cb[:, 5120:5632] = (
        wo.T.reshape(2, P, C).transpose(1, 0, 2).astype(bf).reshape(P, 512)
    )
    cf = np.zeros((P, 12), np.float32)
    cf[:, 0:2] = bg.reshape(2, P).T
    cf[:, 10:12] = bo.reshape(2, P).T

    in_maps = []
    for c in range(NCORES):
        b, qh = c // 2, c % 2
        q0 = qh * QL
        cbc = cb.copy()
        cbc[:, 512:2560] = part_major(kv_x[b].T.astype(bf), K).reshape(P, 2048)
        cbc[:, 3072:4096] = part_major(
            q_x[b, q0:q0 + QL, :].T.astype(bf), QL
        ).reshape(P, 1024)
        cfc = cf.copy()
        uf = np.exp(mask_bias[b, 0, 0]).astype(np.float32).reshape(NKC, P).T
        cfc[:, 2:2 + NKC] = uf
        cbc[:, 5632:5888] = np.repeat(
            uf.astype(bf).reshape(P, NKC, 1), D, axis=2
        ).reshape(P, NKC * D)
        # pbm[t4, j, p, h4, q]: A*pb for sch units, exp(pb) for ACT units
        x = pair_bias[b, :, q0:q0 + QL, :]  # [8h, 512q, 1024k]
        xr = np.ascontiguousarray(
            x.reshape(2, 4, QL, NKC, P).transpose(0, 3, 4, 1, 2)
        ).astype(np.float64)  # [t4, j, p, h4, q]
        pbm = np.empty((2, NKC, P, 4, QL), bf)
        for t4 in range(2):
            for j in range(NKC):
                for pair in range(2):
                    hs = slice(2 * pair, 2 * pair + 2)
                    blk = xr[t4, j, :, hs, :]
                    if _sch_unit(t4, j, pair):
                        pbm[t4, j, :, hs, :] = (SCH_A * blk).astype(bf)
                    else:
                        pbm[t4, j, :, hs, :] = np.exp(blk).astype(bf)
        in_maps.append({"pbm": pbm, "cb": cbc, "cf": cfc})
    return in_maps


def kernel(q_x, kv_x, mask_bias, pair_bias, wq, wk, wv, wg, bg, wo, bo):
    global LAST_RESULTS
    from concourse.bass_utils import run_bass_kernel_spmd

    nc = _get_nc()
    in_maps = _make_in_maps(
        q_x, kv_x, mask_bias, pair_bias, wq, wk, wv, wg, bg, wo, bo
    )
    res = run_bass_kernel_spmd(nc, in_maps, core_ids=list(range(NCORES)))
    LAST_RESULTS = res

    out = np.empty((B, Q, C), np.float32)
    for c in range(NCORES):
        b, qh = c // 2, c % 2
        out[b, qh * QL:(qh + 1) * QL, :] = res.results[c]["out"].T
    return out


# revision 16
# speedup vs baseline: 1.0318x; 1.0318x over previous
"""Distributed TRN2 Bass kernel for OpenFold-style gated attention with pair bias.

Problem: B=4, Q=K=1024, H=8 heads, D=32, C=256 (all fp32):
    q = (q_x @ wq.T)/sqrt(D);  k = kv_x @ wk.T;  v = kv_x @ wv.T
    a = softmax(q k^T + mask_bias + pair_bias)   (softmax over K)
    o = (a v) * sigmoid(q_x @ wg.T + bg)
    out = o @ wo.T + bo

Sharding: 8 cores = (batch b, query-half qh); no collectives; host
concatenates per-core outputs.

v3 dataflow (per core; all feature-on-partitions, no on-device transposes):
  - the PE computes ONLY q.k^T scores (transposed, [k-part, q-free]);
    pair_bias never touches the tensor engine;
  - exp(qk + pb) is factorized exp(qk)*exp(pb) and split per
    (t4, j, pair) unit between two paths:
      * sch units (DVE): pair_bias ships as A*pb bf16 and one fused
        affine_then_add computes int16(round(A*qk + B + A*pb)) whose bits
        ARE the bf16 Schraudolph approximation of exp(qk+pb) -- one DVE
        op per unit, pb-add included;
      * ACT units: pair_bias ships as exp(pb) bf16; ACT's hardware Exp
        LUT computes exp(qk) (|qk| <~ 0.6) and an all-bf16 multiply
        (GpSimd or DVE at 2x rate) forms exp(qk)*exp(pb);
    the per-unit choice is a compile-time schedule and the host packs the
    matching pb encoding into one [t4, j, p, h4, q] bf16 tensor, streamed
    on both hardware DGE queues (sync + scalar);
  - AV + denominator matmuls column-packed (denominator via
    32x-replicated u = exp(mask_bias) lhsT, so normalization is pure
    elementwise; den consumes the same p tiles, canceling the sch
    approximation's scale error); gating multiplies run on DVE/GpSimd
    straight out of PSUM; output projection at the tail (score PSUM pool
    triple-buffered during the loop).
"""

import numpy as np

H, D, C = 8, 32, 256
B, Q, K = 4, 1024, 1024
QL = 512  # queries per core
NCORES = 8
P = 128
NKC = K // P  # 8 k-chunks of 128

SCH_A = 184.6649652337873  # 128 * log2(e)
SCH_B = 16248.8


def _sch_unit(t4, j, pair):
    """True -> DVE schraudolph unit (12 of 32), else ACT exp * mul unit."""
    if (j + pair) % 2 == 0:
        return False
    return j not in (3, 5)


def _mul_on_dve(n):
    """For ACT units: 14 of 20 multiplies on DVE (2x bf16), rest on GpSimd."""
    return n % 10 not in (0, 3, 6)


_CACHE = {}
LAST_RESULTS = None


def _build_nc():
    from contextlib import ExitStack

    from concourse import bacc, mybir, tile

    f32 = mybir.dt.float32
    bf16 = mybir.dt.bfloat16
    i16 = mybir.dt.int16
    EXP = mybir.ActivationFunctionType.Exp
    SIG = mybir.ActivationFunctionType.Sigmoid
    IDENT = mybir.ActivationFunctionType.Identity

    nc = bacc.Bacc("TRN2", target_bir_lowering=False, debug=False, num_devices=NCORES)

    CB = 5888  # bf16 constant-blob columns
    pbm_d = nc.dram_tensor("pbm", [2, NKC, P, 4, QL], bf16, kind="ExternalInput").ap()
    cb_d = nc.dram_tensor("cb", [P, CB], bf16, kind="ExternalInput").ap()
    cf_d = nc.dram_tensor("cf", [P, 12], f32, kind="ExternalInput").ap()
    out_d = nc.dram_tensor("out", [C, QL], f32, kind="ExternalOutput").ap()

    with tile.TileContext(nc) as tc, ExitStack() as ctx:
        # ---- persistent tiles -------------------------------------------
        cp = ctx.enter_context(tc.tile_pool(name="const", bufs=1))

        def ptile(shape, dtype, name):
            return cp.tile(shape, dtype, name=name, tag=name)

        cb_sb = ptile([P, CB], bf16, "cb_sb")
        cf_sb = ptile([P, 12], f32, "cf_sb")

        def cbv(lo, hi, a=None):
            v = cb_sb[:, lo:hi]
            return v.rearrange("p (a b) -> p a b", a=a) if a else v

        wk_bf = cbv(0, 512, 2)        # [128, 2, 256]
        kv_bf = cbv(512, 2560, 2)     # [128, 2, 1024]
        wq_bf = cbv(2560, 3072, 2)
        qx_bf = cbv(3072, 4096, 2)    # [128, 2, 512]
        wv_bf = cbv(4096, 4608, 2)
        wg_bf = cbv(4608, 5120, 2)
        woB_bf = cbv(5120, 5632, 2)   # [hd-in-half, half t4, c]
        u32_bf = cbv(5632, 5888, NKC)  # [128, 8, 32]: u replicated 32x
        bgT_sb = cf_sb[:, 0:2]
        u_sb = cf_sb[:, 2:2 + NKC]  # f32 u = exp(mask_bias), host-computed
        boT_sb = cf_sb[:, 10:12]

        qT_bf = ptile([P, 2, QL], bf16, "qT_bf")  # [hd-part, t, q]
        kT_bf = ptile([P, 2, K], bf16, "kT_bf")  # [hd-part, t, k]
        v1_bf = ptile([P, NKC, C], bf16, "v1_bf")  # v * u, [k-part, chunk, hd]
        g_bf = ptile([P, 2, QL], bf16, "g_bf")  # sigmoid gate, stacked halves
        og_bf = ptile([P, 2, QL], bf16, "og_bf")  # gated+normalized o^T

        # constants split across all three DMA queues by priority; pair-bias
        # tiles round-robin the same three queues (aggregate BW ~2.5x single
        # queue). Rolling prefetch 3 deep (pool bufs=4).
        nc.sync.dma_start(out=cb_sb[:, 0:2560], in_=cb_d[:, 0:2560])
        nc.scalar.dma_start(out=cb_sb[:, 2560:4096], in_=cb_d[:, 2560:4096])
        nc.gpsimd.dma_start(out=cb_sb[:, 4096:CB], in_=cb_d[:, 4096:CB])
        nc.gpsimd.dma_start(out=cf_sb[:], in_=cf_d[:])

        pb_pool = ctx.enter_context(tc.tile_pool(name="pb", bufs=4))
        pb_order = [(t4, j) for t4 in range(2) for j in range(NKC)]
        pbt = {}
        pb_engs = [nc.sync, nc.scalar, nc.gpsimd]

        def issue_pb(idx):
            if idx < len(pb_order):
                t4, j = pb_order[idx]
                t = pb_pool.tile([P, 4, QL], bf16, tag="pb", name=f"pb{t4}_{j}")
                pb_engs[idx % 3].dma_start(out=t[:], in_=pbm_d[t4, j])
                pbt[(t4, j)] = t

        for i in range(3):
            issue_pb(i)

        # ---- stage 1: projections ---------------------------------------
        with tc.tile_pool(name="ps1", bufs=3, space="PSUM") as ps1:
            # kT[f, k] first (the attention critical path), then qT[f, q]
            for t in range(2):
                for fc in range(2):
                    ps = ps1.tile([P, QL], f32, tag="ps1")
                    for ci in range(2):
                        nc.tensor.matmul(
                            ps[:],
                            lhsT=wk_bf[:, ci, t * P:(t + 1) * P],
                            rhs=kv_bf[:, ci, fc * QL:(fc + 1) * QL],
                            start=(ci == 0),
                            stop=(ci == 1),
                        )
                    nc.scalar.activation(
                        kT_bf[:, t, fc * QL:(fc + 1) * QL], ps[:],
                        mybir.ActivationFunctionType.Copy,
                    )

            for t in range(2):
                ps = ps1.tile([P, QL], f32, tag="ps1")
                for ci in range(2):
                    nc.tensor.matmul(
                        ps[:],
                        lhsT=wq_bf[:, ci, t * P:(t + 1) * P],
                        rhs=qx_bf[:, ci, :],
                        start=(ci == 0),
                        stop=(ci == 1),
                    )
                nc.vector.tensor_copy(qT_bf[:, t, :], ps[:])

            # v per k-chunk, scaled per-partition by u = exp(mask_bias)
            for j in range(NKC):
                ps = ps1.tile([P, C], f32, tag="ps1")
                for ci in range(2):
                    nc.tensor.matmul(
                        ps[:],
                        lhsT=kv_bf[:, ci, j * P:(j + 1) * P],
                        rhs=wv_bf[:, ci, :],
                        start=(ci == 0),
                        stop=(ci == 1),
                    )
                nc.scalar.activation(
                    v1_bf[:, j, :], ps[:], mybir.ActivationFunctionType.Copy,
                    bias=0.0, scale=u_sb[:, j:j + 1],
                )

            # gate halves: g = sigmoid(wg x + bg), stacked [128=4 heads x 32d]
            for t in range(2):
                ps = ps1.tile([P, QL], f32, tag="ps1")
                for ci in range(2):
                    nc.tensor.matmul(
                        ps[:],
                        lhsT=wg_bf[:, ci, t * P:(t + 1) * P],
                        rhs=qx_bf[:, ci, :],
                        start=(ci == 0),
                        stop=(ci == 1),
                    )
                nc.scalar.activation(
                    g_bf[:, t, :], ps[:], SIG, bias=bgT_sb[:, t:t + 1]
                )

        # ---- stage 2: attention, 2 groups of 4 column-packed heads ------
        mul_n = 0
        with tc.tile_pool(name="pp", bufs=4) as p_pool, tc.tile_pool(
            name="eq", bufs=3
        ) as eq_pool, tc.tile_pool(name="nrm", bufs=2) as nrm, tc.tile_pool(
            name="ps_s", bufs=3, space="PSUM"
        ) as ps_s, tc.tile_pool(name="ps_o", bufs=1, space="PSUM") as ps_o, tc.tile_pool(
            name="ps_d", bufs=1, space="PSUM"
        ) as ps_d:
            for t4 in range(2):
                o_ps = ps_o.tile([P, QL], f32, tag="ps_o")
                d_ps = ps_d.tile([P, QL], f32, tag="ps_d")
                p2s = {}

                def emit_av(pj):
                    pp2 = p2s[pj]
                    for h4 in range(4):
                        co = h4 * D
                        nc.tensor.matmul(
                            o_ps[co:co + D, :],
                            lhsT=v1_bf[
                                :, pj, (t4 * 4 + h4) * D:(t4 * 4 + h4 + 1) * D
                            ],
                            rhs=pp2[:, h4, :],
                            start=(pj == 0),
                            stop=(pj == NKC - 1),
                            tile_position=(0, co),
                            skip_group_check=True,
                        )
                    for h4 in range(4):
                        co = h4 * D
                        nc.tensor.matmul(
                            d_ps[co:co + D, :],
                            lhsT=u32_bf[:, pj, :],
                            rhs=pp2[:, h4, :],
                            start=(pj == 0),
                            stop=(pj == NKC - 1),
                            tile_position=(0, co),
                            skip_group_check=True,
                        )

                for j in range(NKC):
                    issue_pb(pb_order.index((t4, j)) + 3)
                    pt = pbt[(t4, j)]
                    p2 = p_pool.tile([P, 4, QL], bf16, tag="p2")
                    p2s[j] = p2
                    for pair in range(2):
                        s2 = ps_s.tile([P, 2, QL], f32, tag="ps_s")
                        for hh in range(2):
                            h = 2 * pair + hh
                            pr = h * D
                            nc.tensor.matmul(
                                s2[:, hh, :],
                                lhsT=kT_bf[pr:pr + D, t4, j * P:(j + 1) * P],
                                rhs=qT_bf[pr:pr + D, t4, :],
                                start=True,
                                stop=True,
                                tile_position=(pr, 0),
                                skip_group_check=True,
                            )
                        pslice = p2[:, 2 * pair:2 * pair + 2, :]
                        pbslice = pt[:, 2 * pair:2 * pair + 2, :]
                        if _sch_unit(t4, j, pair):
                            nc.vector.affine_then_add(
                                pslice.bitcast(i16), s2[:], pbslice,
                                SCH_A, SCH_B,
                            )
                        else:
                            eqk = eq_pool.tile([P, 2, QL], bf16, tag="eqk")
                            nc.scalar.activation(eqk[:], s2[:], EXP)
                            eng = nc.vector if _mul_on_dve(mul_n) else nc.gpsimd
                            mul_n += 1
                            eng.tensor_mul(pslice, eqk[:], pbslice)
                    if j >= 2:
                        emit_av(j - 2)  # 2-deep: absorbs the exp+mul latency
                emit_av(NKC - 2)
                emit_av(NKC - 1)
                # normalize + gate: rb = 1/den (DVE, from PSUM); ge = g*rb
                # (gpsimd, SBUF-only); og = o_ps * ge (DVE, from PSUM)
                rbt = nrm.tile([P, QL], f32, tag="rbt")
                nc.vector.reciprocal_approx_fast(rbt[:], d_ps[:])
                ge = nrm.tile([P, QL], bf16, tag="ge")
                # t4=1's ge is on the exposed tail: keep the whole chain on DVE
                geng = nc.gpsimd if t4 == 0 else nc.vector
                geng.tensor_mul(ge[:], g_bf[:, t4, :], rbt[:])
                nc.vector.tensor_mul(og_bf[:, t4, :], o_ps[:], ge[:])

        # ---- tail: output projection ------------------------------------
        with tc.tile_pool(name="ps_out", bufs=2, space="PSUM") as ps_out_pool, \
                tc.tile_pool(name="otail", bufs=2) as otail:
            for t in range(2):
                ps = ps_out_pool.tile([P, QL], f32, tag="ps_out", name=f"po{t}")
                for t4 in range(2):
                    nc.tensor.matmul(
                        ps[:],
                        lhsT=woB_bf[:, t4, t * P:(t + 1) * P],
                        rhs=og_bf[:, t4, :],
                        start=(t4 == 0),
                        stop=(t4 == 1),
                    )
                o_out = otail.tile([P, QL], f32, tag="o_out", name=f"oo{t}")
                nc.scalar.activation(
                    o_out[:], ps[:], IDENT, bias=boT_sb[:, t:t + 1]
                )
                nc.sync.dma_start(out=out_d[t * P:(t + 1) * P, :], in_=o_out[:])

    nc.compile()
    return nc


def _get_nc():
    if "nc" not in _CACHE:
        _CACHE["nc"] = _build_nc()
    return _CACHE["nc"]


def _make_in_maps(q_x, kv_x, mask_bias, pair_bias, wq, wk, wv, wg, bg, wo, bo):
    f = np.float32
    q_x = np.asarray(q_x, f)
    kv_x = np.asarray(kv_x, f)
    mask_bias = np.asarray(mask_bias, f)
    pair_bias = np.asarray(pair_bias, f)
    wq = np.asarray(wq, f)
    wk = np.asarray(wk, f)
    wv = np.asarray(wv, f)
    wg = np.asarray(wg, f)
    bg = np.asarray(bg, f)
    wo = np.asarray(wo, f)
    bo = np.asarray(bo, f)

    import ml_dtypes
    bf = ml_dtypes.bfloat16

    def part_major(x, cols):  # [256, cols] -> [128, 2, cols] partition-major
        return x.reshape(2, P, cols).transpose(1, 0, 2)

    CB = 5888
    cb = np.zeros((P, CB), bf)
    cb[:, 0:512] = part_major(wk.T.astype(bf), C).reshape(P, 512)
    cb[:, 2560:3072] = part_major((wq / np.sqrt(D)).T.astype(bf), C).reshape(P, 512)
    cb[:, 4096:4608] = part_major(wv.T.astype(bf), C).reshape(P, 512)
    cb[:, 4608:5120] = part_major(wg.T.astype(bf), C).reshape(P, 512)
    cb[:, 5120:5632] = (
        wo.T.reshape(2, P, C).transpose(1, 0, 2).astype(bf).reshape(P, 512)
    )
    cf = np.zeros((P, 12), np.float32)
    cf[:, 0:2] = bg.reshape(2, P).T
    cf[:, 10:12] = bo.reshape(2, P).T

    in_maps = []
    for c in range(NCORES):
        b, qh = c // 2, c % 2
        q0 = qh * QL
        cbc = cb.copy()
        cbc[:, 512:2560] = part_major(kv_x[b].T.astype(bf), K).reshape(P, 2048)
        cbc[:, 3072:4096] = part_major(
            q_x[b, q0:q0 + QL, :].T.astype(bf), QL
        ).reshape(P, 1024)
        cfc = cf.copy()
        uf = np.exp(mask_bias[b, 0, 0]).astype(np.float32).reshape(NKC, P).T
        cfc[:, 2:2 + NKC] = uf
        cbc[:, 5632:5888] = np.repeat(
            uf.astype(bf).reshape(P, NKC, 1), D, axis=2
        ).reshape(P, NKC * D)
        # pbm[t4, j, p, h4, q]: A*pb for sch units, exp(pb) for ACT units
        x = pair_bias[b, :, q0:q0 + QL, :]  # [8h, 512q, 1024k]
        xr = np.ascontiguousarray(
            x.reshape(2, 4, QL, NKC, P).transpose(0, 3, 4, 1, 2)
        ).astype(np.float64)  # [t4, j, p, h4, q]
        pbm = np.empty((2, NKC, P, 4, QL), bf)
        for t4 in range(2):
            for j in range(NKC):
                for pair in range(2):
                    hs = slice(2 * pair, 2 * pair + 2)
                    blk = xr[t4, j, :, hs, :]
                    if _sch_unit(t4, j, pair):
                        pbm[t4, j, :, hs, :] = (SCH_A * blk).astype(bf)
                    else:
                        pbm[t4, j, :, hs, :] = np.exp(blk).astype(bf)
        in_maps.append({"pbm": pbm, "cb": cbc, "cf": cfc})
    return in_maps


def kernel(q_x, kv_x, mask_bias, pair_bias, wq, wk, wv, wg, bg, wo, bo):
    global LAST_RESULTS
    from concourse.bass_utils import run_bass_kernel_spmd

    nc = _get_nc()
    in_maps = _make_in_maps(
        q_x, kv_x, mask_bias, pair_bias, wq, wk, wv, wg, bg, wo, bo
    )
    res = run_bass_kernel_spmd(nc, in_maps, core_ids=list(range(NCORES)))
    LAST_RESULTS = res

    out = np.empty((B, Q, C), np.float32)
    for c in range(NCORES):
        b, qh = c // 2, c % 2
        out[b, qh * QL:(qh + 1) * QL, :] = res.results[c]["out"].T
    return out


# revision 18
# speedup vs baseline: 1.0528x; 1.0204x over previous
"""Distributed TRN2 Bass kernel for OpenFold-style gated attention with pair bias.

Problem: B=4, Q=K=1024, H=8 heads, D=32, C=256 (all fp32):
    q = (q_x @ wq.T)/sqrt(D);  k = kv_x @ wk.T;  v = kv_x @ wv.T
    a = softmax(q k^T + mask_bias + pair_bias)   (softmax over K)
    o = (a v) * sigmoid(q_x @ wg.T + bg)
    out = o @ wo.T + bo

Sharding: 8 cores = (batch b, query-half qh); no collectives; host
concatenates per-core outputs.

v3 dataflow (per core; all feature-on-partitions, no on-device transposes):
  - the PE computes ONLY q.k^T scores (transposed, [k-part, q-free]);
    pair_bias never touches the tensor engine;
  - exp(qk + pb) is factorized exp(qk)*exp(pb) and split per
    (t4, j, pair) unit between two paths:
      * sch units (DVE): pair_bias ships as A*pb bf16 and one fused
        affine_then_add computes int16(round(A*qk + B + A*pb)) whose bits
        ARE the bf16 Schraudolph approximation of exp(qk+pb) -- one DVE
        op per unit, pb-add included;
      * ACT units: pair_bias ships as exp(pb) bf16; ACT's hardware Exp
        LUT computes exp(qk) (|qk| <~ 0.6) and an all-bf16 multiply
        (GpSimd or DVE at 2x rate) forms exp(qk)*exp(pb);
    the per-unit choice is a compile-time schedule and the host packs the
    matching pb encoding into one [t4, j, p, h4, q] bf16 tensor, streamed
    on both hardware DGE queues (sync + scalar);
  - AV + denominator matmuls column-packed (denominator via
    32x-replicated u = exp(mask_bias) lhsT, so normalization is pure
    elementwise; den consumes the same p tiles, canceling the sch
    approximation's scale error); gating multiplies run on DVE/GpSimd
    straight out of PSUM; output projection at the tail (score PSUM pool
    triple-buffered during the loop).
"""

import numpy as np

H, D, C = 8, 32, 256
B, Q, K = 4, 1024, 1024
QL = 512  # queries per core
NCORES = 8
P = 128
NKC = K // P  # 8 k-chunks of 128

SCH_A = 184.6649652337873  # 128 * log2(e)
SCH_B = 16248.8


def _sch_unit(t4, j, pair):
    """True -> DVE schraudolph unit (12 of 32), else ACT exp * mul unit."""
    if (j + pair) % 2 == 0:
        return False
    return j not in (3, 5)


def _mul_on_dve(n):
    """For ACT units: 14 of 20 multiplies on DVE (2x bf16), rest on GpSimd."""
    return n % 10 not in (0, 3, 6)


_CACHE = {}
LAST_RESULTS = None


def _build_nc():
    from contextlib import ExitStack

    from concourse import bacc, mybir, tile

    f32 = mybir.dt.float32
    bf16 = mybir.dt.bfloat16
    i16 = mybir.dt.int16
    EXP = mybir.ActivationFunctionType.Exp
    SIG = mybir.ActivationFunctionType.Sigmoid
    IDENT = mybir.ActivationFunctionType.Identity

    nc = bacc.Bacc("TRN2", target_bir_lowering=False, debug=False, num_devices=NCORES)

    CB = 5888  # bf16 constant-blob columns
    pbm_d = nc.dram_tensor("pbm", [2, NKC, P, 4, QL], bf16, kind="ExternalInput").ap()
    cb_d = nc.dram_tensor("cb", [P, CB], bf16, kind="ExternalInput").ap()
    cf_d = nc.dram_tensor("cf", [P, 12], f32, kind="ExternalInput").ap()
    out_d = nc.dram_tensor("out", [C, QL], f32, kind="ExternalOutput").ap()

    with tile.TileContext(nc) as tc, ExitStack() as ctx:
        # ---- persistent tiles -------------------------------------------
        cp = ctx.enter_context(tc.tile_pool(name="const", bufs=1))

        def ptile(shape, dtype, name):
            return cp.tile(shape, dtype, name=name, tag=name)

        cb_sb = ptile([P, CB], bf16, "cb_sb")
        cf_sb = ptile([P, 12], f32, "cf_sb")

        def cbv(lo, hi, a=None):
            v = cb_sb[:, lo:hi]
            return v.rearrange("p (a b) -> p a b", a=a) if a else v

        wk_bf = cbv(0, 512, 2)        # [128, 2, 256]
        kv_bf = cbv(512, 2560, 2)     # [128, 2, 1024]
        wq_bf = cbv(2560, 3072, 2)
        qx_bf = cbv(3072, 4096, 2)    # [128, 2, 512]
        wv_bf = cbv(4096, 4608, 2)
        wg_bf = cbv(4608, 5120, 2)
        woB_bf = cbv(5120, 5632, 2)   # [hd-in-half, half t4, c]
        u32_bf = cbv(5632, 5888, NKC)  # [128, 8, 32]: u replicated 32x
        bgT_sb = cf_sb[:, 0:2]
        u_sb = cf_sb[:, 2:2 + NKC]  # f32 u = exp(mask_bias), host-computed
        boT_sb = cf_sb[:, 10:12]

        qT_bf = ptile([P, 2, QL], bf16, "qT_bf")  # [hd-part, t, q]
        kT_bf = ptile([P, 2, K], bf16, "kT_bf")  # [hd-part, t, k]
        v1_bf = ptile([P, NKC, C], bf16, "v1_bf")  # v * u, [k-part, chunk, hd]
        g_bf = ptile([P, 2, QL], bf16, "g_bf")  # sigmoid gate, stacked halves
        og_bf = ptile([P, 2, QL], bf16, "og_bf")  # gated+normalized o^T

        # constants split across all three DMA queues by priority; pair-bias
        # tiles round-robin the same three queues (aggregate BW ~2.5x single
        # queue). Rolling prefetch 3 deep (pool bufs=4).
        nc.sync.dma_start(out=cb_sb[:, 0:2560], in_=cb_d[:, 0:2560])
        nc.scalar.dma_start(out=cb_sb[:, 2560:4096], in_=cb_d[:, 2560:4096])
        nc.gpsimd.dma_start(out=cb_sb[:, 4096:CB], in_=cb_d[:, 4096:CB])
        nc.gpsimd.dma_start(out=cf_sb[:], in_=cf_d[:])

        # 2-j slabs halve trigger overhead; spread across the three queues
        pb_pool = ctx.enter_context(tc.tile_pool(name="pb", bufs=3))
        pb_slabs = [(t4, j2) for t4 in range(2) for j2 in range(NKC // 2)]
        pbt = {}
        pb_engs = [nc.sync, nc.scalar, nc.sync, nc.gpsimd,
                   nc.sync, nc.scalar, nc.sync, nc.gpsimd]

        def issue_pb_slab(idx):
            if idx < len(pb_slabs):
                t4, j2 = pb_slabs[idx]
                t = pb_pool.tile(
                    [P, 2, 4, QL], bf16, tag="pb", name=f"pb{t4}_{j2}"
                )
                pb_engs[idx].dma_start(
                    out=t[:],
                    in_=pbm_d[t4, 2 * j2:2 * j2 + 2].rearrange(
                        "j p h q -> p j h q"
                    ),
                )
                pbt[(t4, 2 * j2)] = t[:, 0]
                pbt[(t4, 2 * j2 + 1)] = t[:, 1]

        for i in range(2):
            issue_pb_slab(i)

        # ---- stage 1: projections ---------------------------------------
        with tc.tile_pool(name="ps1", bufs=3, space="PSUM") as ps1:
            # kT[f, k] first (the attention critical path), then qT[f, q]
            for t in range(2):
                for fc in range(2):
                    ps = ps1.tile([P, QL], f32, tag="ps1")
                    for ci in range(2):
                        nc.tensor.matmul(
                            ps[:],
                            lhsT=wk_bf[:, ci, t * P:(t + 1) * P],
                            rhs=kv_bf[:, ci, fc * QL:(fc + 1) * QL],
                            start=(ci == 0),
                            stop=(ci == 1),
                        )
                    nc.scalar.activation(
                        kT_bf[:, t, fc * QL:(fc + 1) * QL], ps[:],
                        mybir.ActivationFunctionType.Copy,
                    )

            for t in range(2):
                ps = ps1.tile([P, QL], f32, tag="ps1")
                for ci in range(2):
                    nc.tensor.matmul(
                        ps[:],
                        lhsT=wq_bf[:, ci, t * P:(t + 1) * P],
                        rhs=qx_bf[:, ci, :],
                        start=(ci == 0),
                        stop=(ci == 1),
                    )
                nc.vector.tensor_copy(qT_bf[:, t, :], ps[:])

            # v per k-chunk, scaled per-partition by u = exp(mask_bias)
            for j in range(NKC):
                ps = ps1.tile([P, C], f32, tag="ps1")
                for ci in range(2):
                    nc.tensor.matmul(
                        ps[:],
                        lhsT=kv_bf[:, ci, j * P:(j + 1) * P],
                        rhs=wv_bf[:, ci, :],
                        start=(ci == 0),
                        stop=(ci == 1),
                    )
                nc.scalar.activation(
                    v1_bf[:, j, :], ps[:], mybir.ActivationFunctionType.Copy,
                    bias=0.0, scale=u_sb[:, j:j + 1],
                )

            # gate halves: g = sigmoid(wg x + bg), stacked [128=4 heads x 32d]
            for t in range(2):
                ps = ps1.tile([P, QL], f32, tag="ps1")
                for ci in range(2):
                    nc.tensor.matmul(
                        ps[:],
                        lhsT=wg_bf[:, ci, t * P:(t + 1) * P],
                        rhs=qx_bf[:, ci, :],
                        start=(ci == 0),
                        stop=(ci == 1),
                    )
                nc.scalar.activation(
                    g_bf[:, t, :], ps[:], SIG, bias=bgT_sb[:, t:t + 1]
                )

        # ---- stage 2: attention, 2 groups of 4 column-packed heads ------
        mul_n = 0
        with tc.tile_pool(name="pp", bufs=4) as p_pool, tc.tile_pool(
            name="eq", bufs=3
        ) as eq_pool, tc.tile_pool(name="nrm", bufs=2) as nrm, tc.tile_pool(
            name="ps_s", bufs=3, space="PSUM"
        ) as ps_s, tc.tile_pool(name="ps_o", bufs=1, space="PSUM") as ps_o, tc.tile_pool(
            name="ps_d", bufs=1, space="PSUM"
        ) as ps_d:
            for t4 in range(2):
                o_ps = ps_o.tile([P, QL], f32, tag="ps_o")
                d_ps = ps_d.tile([P, QL], f32, tag="ps_d")
                p2s = {}

                def emit_av(pj):
                    pp2 = p2s[pj]
                    for h4 in range(4):
                        co = h4 * D
                        nc.tensor.matmul(
                            o_ps[co:co + D, :],
                            lhsT=v1_bf[
                                :, pj, (t4 * 4 + h4) * D:(t4 * 4 + h4 + 1) * D
                            ],
                            rhs=pp2[:, h4, :],
                            start=(pj == 0),
                            stop=(pj == NKC - 1),
                            tile_position=(0, co),
                            skip_group_check=True,
                        )
                    for h4 in range(4):
                        co = h4 * D
                        nc.tensor.matmul(
                            d_ps[co:co + D, :],
                            lhsT=u32_bf[:, pj, :],
                            rhs=pp2[:, h4, :],
                            start=(pj == 0),
                            stop=(pj == NKC - 1),
                            tile_position=(0, co),
                            skip_group_check=True,
                        )

                for j in range(NKC):
                    if j % 2 == 0:
                        issue_pb_slab(pb_slabs.index((t4, j // 2)) + 2)
                    pt = pbt[(t4, j)]
                    p2 = p_pool.tile([P, 4, QL], bf16, tag="p2")
                    p2s[j] = p2
                    for pair in range(2):
                        s2 = ps_s.tile([P, 2, QL], f32, tag="ps_s")
                        for hh in range(2):
                            h = 2 * pair + hh
                            pr = h * D
                            nc.tensor.matmul(
                                s2[:, hh, :],
                                lhsT=kT_bf[pr:pr + D, t4, j * P:(j + 1) * P],
                                rhs=qT_bf[pr:pr + D, t4, :],
                                start=True,
                                stop=True,
                                tile_position=(pr, 0),
                                skip_group_check=True,
                            )
                        pslice = p2[:, 2 * pair:2 * pair + 2, :]
                        pbslice = pt[:, 2 * pair:2 * pair + 2, :]
                        if _sch_unit(t4, j, pair):
                            nc.vector.affine_then_add(
                                pslice.bitcast(i16), s2[:], pbslice,
                                SCH_A, SCH_B,
                            )
                        else:
                            eqk = eq_pool.tile([P, 2, QL], bf16, tag="eqk")
                            nc.scalar.activation(eqk[:], s2[:], EXP)
                            eng = nc.vector if _mul_on_dve(mul_n) else nc.gpsimd
                            mul_n += 1
                            eng.tensor_mul(pslice, eqk[:], pbslice)
                    if j >= 2:
                        emit_av(j - 2)  # 2-deep: absorbs the exp+mul latency
                emit_av(NKC - 2)
                emit_av(NKC - 1)
                # normalize + gate: rb = 1/den (DVE, from PSUM); ge = g*rb
                # (gpsimd, SBUF-only); og = o_ps * ge (DVE, from PSUM)
                rbt = nrm.tile([P, QL], f32, tag="rbt")
                nc.vector.reciprocal_approx_fast(rbt[:], d_ps[:])
                ge = nrm.tile([P, QL], bf16, tag="ge")
                # t4=1's ge is on the exposed tail: keep the whole chain on DVE
                geng = nc.gpsimd if t4 == 0 else nc.vector
                geng.tensor_mul(ge[:], g_bf[:, t4, :], rbt[:])
                nc.vector.tensor_mul(og_bf[:, t4, :], o_ps[:], ge[:])

        # ---- tail: output projection ------------------------------------
        with tc.tile_pool(name="ps_out", bufs=2, space="PSUM") as ps_out_pool, \
                tc.tile_pool(name="otail", bufs=2) as otail:
            for t in range(2):
                ps = ps_out_pool.tile([P, QL], f32, tag="ps_out", name=f"po{t}")
                for t4 in range(2):
                    nc.tensor.matmul(
                        ps[:],
                        lhsT=woB_bf[:, t4, t * P:(t + 1) * P],
                        rhs=og_bf[:, t4, :],
                        start=(t4 == 0),
                        stop=(t4 == 1),
                    )
                o_out = otail.tile([P, QL], f32, tag="o_out", name=f"oo{t}")
                nc.scalar.activation(
                    o_out[:], ps[:], IDENT, bias=boT_sb[:, t:t + 1]
                )
                nc.sync.dma_start(out=out_d[t * P:(t + 1) * P, :], in_=o_out[:])

    nc.compile()
    return nc


def _get_nc():
    if "nc" not in _CACHE:
        _CACHE["nc"] = _build_nc()
    return _CACHE["nc"]


def _make_in_maps(q_x, kv_x, mask_bias, pair_bias, wq, wk, wv, wg, bg, wo, bo):
    f = np.float32
    q_x = np.asarray(q_x, f)
    kv_x = np.asarray(kv_x, f)
    mask_bias = np.asarray(mask_bias, f)
    pair_bias = np.asarray(pair_bias, f)
    wq = np.asarray(wq, f)
    wk = np.asarray(wk, f)
    wv = np.asarray(wv, f)
    wg = np.asarray(wg, f)
    bg = np.asarray(bg, f)
    wo = np.asarray(wo, f)
    bo = np.asarray(bo, f)

    import ml_dtypes
    bf = ml_dtypes.bfloat16

    def part_major(x, cols):  # [256, cols] -> [128, 2, cols] partition-major
        return x.reshape(2, P, cols).transpose(1, 0, 2)

    CB = 5888
    cb = np.zeros((P, CB), bf)
    cb[:, 0:512] = part_major(wk.T.astype(bf), C).reshape(P, 512)
    cb[:, 2560:3072] = part_major((wq / np.sqrt(D)).T.astype(bf), C).reshape(P, 512)
    cb[:, 4096:4608] = part_major(wv.T.astype(bf), C).reshape(P, 512)
    cb[:, 4608:5120] = part_major(wg.T.astype(bf), C).reshape(P, 512)
    cb[:, 5120:5632] = (
        wo.T.reshape(2, P, C).transpose(1, 0, 2).astype(bf).reshape(P, 512)
    )
    cf = np.zeros((P, 12), np.float32)
    cf[:, 0:2] = bg.reshape(2, P).T
    cf[:, 10:12] = bo.reshape(2, P).T

    in_maps = []
    for c in range(NCORES):
        b, qh = c // 2, c % 2
        q0 = qh * QL
        cbc = cb.copy()
        cbc[:, 512:2560] = part_major(kv_x[b].T.astype(bf), K).reshape(P, 2048)
        cbc[:, 3072:4096] = part_major(
            q_x[b, q0:q0 + QL, :].T.astype(bf), QL
        ).reshape(P, 1024)
        cfc = cf.copy()
        uf = np.exp(mask_bias[b, 0, 0]).astype(np.float32).reshape(NKC, P).T
        cfc[:, 2:2 + NKC] = uf
        cbc[:, 5632:5888] = np.repeat(
            uf.astype(bf).reshape(P, NKC, 1), D, axis=2
        ).reshape(P, NKC * D)
        # pbm[t4, j, p, h4, q]: A*pb for sch units, exp(pb) for ACT units
        x = pair_bias[b, :, q0:q0 + QL, :]  # [8h, 512q, 1024k]
        xr = np.ascontiguousarray(
            x.reshape(2, 4, QL, NKC, P).transpose(0, 3, 4, 1, 2)
        ).astype(np.float64)  # [t4, j, p, h4, q]
        pbm = np.empty((2, NKC, P, 4, QL), bf)
        for t4 in range(2):
            for j in range(NKC):
                for pair in range(2):
                    hs = slice(2 * pair, 2 * pair + 2)
                    blk = xr[t4, j, :, hs, :]
                    if _sch_unit(t4, j, pair):
                        pbm[t4, j, :, hs, :] = (SCH_A * blk).astype(bf)
                    else:
                        pbm[t4, j, :, hs, :] = np.exp(blk).astype(bf)
        in_maps.append({"pbm": pbm, "cb": cbc, "cf": cfc})
    return in_maps


def kernel(q_x, kv_x, mask_bias, pair_bias, wq, wk, wv, wg, bg, wo, bo):
    global LAST_RESULTS
    from concourse.bass_utils import run_bass_kernel_spmd

    nc = _get_nc()
    in_maps = _make_in_maps(
        q_x, kv_x, mask_bias, pair_bias, wq, wk, wv, wg, bg, wo, bo
    )
    res = run_bass_kernel_spmd(nc, in_maps, core_ids=list(range(NCORES)))
    LAST_RESULTS = res

    out = np.empty((B, Q, C), np.float32)
    for c in range(NCORES):
        b, qh = c // 2, c % 2
        out[b, qh * QL:(qh + 1) * QL, :] = res.results[c]["out"].T
    return out
